# revision 54
# baseline (speedup 1.0000x reference)
"""Trainium2 Bass kernel for nn_MultiHeadAttention (B=2, S=2048, D=1024, H=16).

Sharding: 8 cores = 2 (batch) x 4 (head groups of 4 heads / 256 dims).
Each core computes QKV projections for its head slice, attention for its 4
heads, and the partial output projection for its 256-dim slice of Wo's input.
Host sums the 4 partials per batch element (Megatron-style row-parallel Wo).

Device layouts (per core; all DRAM inputs pre-tiled on host so every DMA
descriptor covers a full 8-16KB partition line):
  qT/kT/vT  [4(sb), 128, 8, 512] bf16
  wkq       [128, 8, 512] bf16  (Wk[js].T | Wq[js].T interleaved per chunk)
  wvT       [128, 8, 256] bf16; woT [256, 1024] bf16 (Wo[:, js].T)
  maskT     [4(sb), 128, 16, 512] bf16 (mask.T as 0.0/1.0)
  qpS/kpT   [256(j), 2048(s)]   (projections, transposed: j on partitions)
  vp        [2048(t), 4x128]    (per head: 64 dims | 64 ones-cols -> attn@V
                                 emits the softmax denom on 64 psum rows)
  P~        [t, s] = exp(scoresT/8) * maskT   (scoresT = K_h.T^T @ Q_h.T)
  attn out  [128(j+denom), s] -> normalized -> concatT [256(j), 2048(s)]
  out_p     [2048, 1024] bf16 partial = concatT.T @ woT (host sums in f32)

Pipeline: per chunk [attnV(prev) -> scores(cur, 2 heads row-tile-concurrent)
-> exp on ACT]; ACT (143us of exp) is the pacing engine; projections/Wo fill
PE slack via a deadline-placed extras schedule; DMA-engine+HAM warmup and a
fine-grained last iteration minimize ramp and tail.
"""

import sys

import numpy as np

try:
    import concourse.bass as bass
except ImportError:  # pragma: no cover
    sys.path.insert(0, "/opt/trn_rl_repo")
    import concourse.bass as bass

from concourse import bacc

import ml_dtypes

import concourse.tile as tile_mod
from concourse import mybir
from concourse.bass_utils import run_bass_kernel_spmd

BF16 = ml_dtypes.bfloat16
F32 = np.float32

B, S, D, H = 2, 2048, 1024, 16
DK = D // H            # 64
N_CORES = 8
HPC = 4                # heads per core
JC = HPC * DK          # 256 j-dims per core
SCALE = 1.0 / float(np.sqrt(DK))
NSB = S // 512         # 4 s-blocks
NC_T = S // 128        # 16 t-chunks
VROW = HPC * 128       # 512: [h0 64dims | 64 ones | h1 ...]; the 64
                       # ones-columns make attnV emit the softmax denom
                       # replicated on 64 psum partitions (free: matmul
                       # time is column-count of the moving operand)

bf = mybir.dt.bfloat16
f32 = mybir.dt.float32


def _patch_drain():
    """This walrus build only accepts 1 sync-wait per instruction; the Tile
    exit drain carries one wait per pending proc. Split them across drains."""
    if getattr(tile_mod.TileContext, "_drain_patched", False):
        return
    import bass_rust

    def _drain_and_barrier(self, tick_clock, wait_clock):
        from concourse.tile import ScopedClock

        nc = self.nc
        drain_inst = nc.sync.drain()
        wait_clock.add_sem_waits(
            drain_inst.ins, ScopedClock({None: tick_clock.global_clock})
        )
        si = drain_inst.ins.sync_info
        waits = list(si.on_wait)
        if len(waits) > 1:
            drain_inst.ins.sync_info = bass_rust.SyncInfo(
                on_wait=[waits[0]], on_update=list(si.on_update)
            )
            for w in waits[1:]:
                d2 = nc.sync.drain()
                d2.ins.sync_info = bass_rust.SyncInfo(on_wait=[w], on_update=[])
        nc.all_engine_barrier()
        assert self.sems is not None
        popped = nc._tile_sem_poison_stack.pop()
        assert popped is self._sem_poison
        nc.clear_and_free_semaphores(list(self.sems.allocated().values()))
        nc.all_engine_barrier()

    tile_mod.TileContext._drain_and_barrier = _drain_and_barrier
    tile_mod.TileContext._drain_patched = True


def _emit(tc, T):
    nc = tc.nc
    Exp = mybir.ActivationFunctionType.Exp

    from contextlib import ExitStack

    with ExitStack() as ctx:
        persist = ctx.enter_context(tc.tile_pool(name="persist", bufs=1))

        # ---- weights / persistent tiles ----
        # wk and wq live in one tile and arrive in one DMA (per-queue DMA
        # completions release at a ~2.5us-per-instruction cadence, so the
        # startup-critical path wants the fewest possible instructions)
        wkq = persist.tile([128, 8 * 2 * JC], bf, tag="wkq")
        wv = persist.tile([128, 8 * JC], bf, tag="wv")
        wo = [persist.tile([128, D], bf, tag=f"wo{i}", name=f"wo{i}") for i in range(2)]
        biasqk = persist.tile([128, 4], f32, tag="biasqk")

        # The HWDGE (sync-queue) path sustains only ~30-60 GB/s per
        # instruction; the SWDGE (gpsimd-queue) path measures ~150-200 GB/s.
        # Startup-critical transfers go on gpsimd, slack ones on sync.
        def emit_wdma(t, name, eng):
            # host pre-tiles weights as [128, 8, JC] so each partition's
            # 8*JC*2B run is contiguous (big DMA descriptors)
            eng.dma_start(
                t[:].rearrange("p (c j) -> p c j", c=8),
                T[name][:, :, :],
            )

        def emit_wodma(i):
            nc.sync.dma_start(wo[i][:], T["woT"][i * 128 : (i + 1) * 128, :])

        # per-sb q/k projection tiles ([j, s] transposed layout)
        qpS = [
            [persist.tile([128, 512], bf, tag=f"qp{j}_{s}", name=f"qp{j}_{s}")
             for s in range(NSB)]
            for j in range(2)
        ]
        kpT = [
            [persist.tile([128, 1024], bf, tag=f"kpT{i}_{th}", name=f"kpT{i}_{th}")
             for th in range(2)]
            for i in range(2)
        ]
        # per-chunk v tiles (natural [t, j] layout + ones cols)
        vpc = [persist.tile([128, VROW], bf, tag=f"vp{c}", name=f"vp{c}")
               for c in range(NC_T)]
        concatT = [persist.tile([128, S], bf, tag=f"concatT{i}", name=f"concatT{i}") for i in range(2)]

        wkq_v = wkq[:].rearrange("p (c j) -> p c j", c=8)
        wv_v = wv[:].rearrange("p (c j) -> p c j", c=8)

        q_stream = ctx.enter_context(tc.tile_pool(name="q_stream", bufs=1))
        kv_stream = ctx.enter_context(tc.tile_pool(name="kv_stream", bufs=3))
        vstream = ctx.enter_context(tc.tile_pool(name="vstream", bufs=2))
        maskp = ctx.enter_context(tc.tile_pool(name="maskp", bufs=2))
        ptp = ctx.enter_context(tc.tile_pool(name="ptp", bufs=2))
        smallp = ctx.enter_context(tc.tile_pool(name="smallp", bufs=2))
        outp = ctx.enter_context(tc.tile_pool(name="outp", bufs=1))
        scp = ctx.enter_context(tc.tile_pool(name="scp", bufs=2, space="PSUM"))
        bigp = ctx.enter_context(tc.tile_pool(name="bigp", bufs=4, space="PSUM"))
        mtiles = {}
        qtts = {}
        ktts = {}
        vtts = {}
        otiles = {}

        def emit_qdma(sb, eng=None):
            qTt = q_stream.tile([128, 8 * 512], bf, tag="qTt", name=f"qTt{sb}")
            (eng or nc.sync).dma_start(
                qTt[:].rearrange("p (c s) -> p c s", c=8),
                T["qT"][sb, :, :, :],
            )
            qtts[sb] = qTt[:].rearrange("p (c s) -> p c s", c=8)

        def emit_qproj_jt(sb, jt):
            jsl = slice(JC + jt * 128, JC + (jt + 1) * 128)
            ps = bigp.tile([128, 512], f32, tag="big", name=f"pq{sb}_{jt}")
            for c in range(8):
                nc.tensor.matmul(
                    ps[:], wkq_v[:, c, jsl], qtts[sb][:, c, :],
                    start=(c == 0), stop=(c == 7),
                )
            nc.vector.tensor_scalar_add(
                qpS[jt][sb][:], ps[:], biasqk[:, jt : jt + 1]
            )

        def emit_kdma(sb, eng=None):
            kTt = kv_stream.tile([128, 8 * 512], bf, tag="kTt", name=f"kTt{sb}")
            (eng or nc.gpsimd).dma_start(
                kTt[:].rearrange("p (c s) -> p c s", c=8),
                T["kT"][sb, :, :, :],
            )
            ktts[sb] = kTt[:].rearrange("p (c s) -> p c s", c=8)

        def emit_kproj_jt(sb, jt):
            jsl = slice(jt * 128, (jt + 1) * 128)
            ps = bigp.tile([128, 512], f32, tag="big", name=f"pk{sb}_{jt}")
            for c in range(8):
                nc.tensor.matmul(
                    ps[:], wkq_v[:, c, jsl], ktts[sb][:, c, :],
                    start=(c == 0), stop=(c == 7),
                )
            nc.vector.tensor_scalar_add(
                kpT[jt][sb // 2][:, (sb % 2) * 512 : (sb % 2 + 1) * 512],
                ps[:], biasqk[:, 2 + jt : 3 + jt]
            )

        def emit_mask_dma(sb, hf):
            mT = maskp.tile([128, 8 * 512], bf, tag="mT", name=f"mT{sb}_{hf}")
            nc.gpsimd.dma_start(
                mT[:].rearrange("p (c s) -> p c s", c=8),
                T["maskT"][sb, :, hf * 8 : (hf + 1) * 8, :],
            )
            mtiles[(sb, hf)] = mT

        def emit_vdma(tb, eng=None):
            vTt = vstream.tile([128, 8 * 512], bf, tag="vTt", name=f"vTt{tb}")
            (eng or nc.gpsimd).dma_start(
                vTt[:].rearrange("p (c s) -> p c s", c=8),
                T["vT"][tb, :, :, :],
            )
            vtts[tb] = vTt[:].rearrange("p (c t) -> p c t", c=8)

        def emit_vproj(chunk):
            tb, tt = chunk // 4, chunk % 4
            vTt_v = vtts[tb]
            ps = bigp.tile([128, 512], f32, tag="big", name=f"pv{chunk}")
            for c in range(8):
                nc.tensor.matmul(
                    ps[:, 0:JC],
                    vTt_v[:, c, tt * 128 : (tt + 1) * 128],
                    wv_v[:, c, :],
                    start=(c == 0), stop=(c == 7),
                )
            vt = vpc[chunk]
            nc.gpsimd.memset(
                vt[:].rearrange("p (h d) -> p h d", d=128)[:, :, 64:128],
                1.0,
            )
            dst = vt[:].rearrange("p (h d) -> p h d", h=HPC)[:, :, 0:DK]
            src = ps[:, 0:JC].rearrange("p (h d) -> p h d", h=HPC)
            nc.vector.tensor_copy(dst, src)

        def emit_wo_group(sb, st, mt):
            # out partial in bf16 (summed in fp32 on host); all 8 groups of
            # an s-block accumulate into one wide tile, flushed by a single
            # 1MB DMA on the fast gpsimd queue when the last group lands.
            s0 = sb * 512 + st * 128
            msl = slice(mt * 512, (mt + 1) * 512)
            pw = bigp.tile([128, 512], f32, tag="big", name=f"pw{sb}_{st}_{mt}")
            for kc in range(2):
                nc.tensor.matmul(
                    pw[:],
                    concatT[kc][:, s0 : s0 + 128],
                    wo[kc][:, msl],
                    start=(kc == 0), stop=(kc == 1),
                )
            if (st, mt) == (0, 0):
                otiles[sb] = outp.tile(
                    [128, 4096], bf, tag="ot", name=f"ot{sb}"
                )
            ot = otiles[sb]
            dst = ot[:, st * 1024 + mt * 512 : st * 1024 + (mt + 1) * 512]
            if sb == NSB - 1 and mt == 0:
                nc.scalar.copy(dst, pw[:])   # tail: ACT is idle, split load
            else:
                nc.vector.tensor_copy(dst, pw[:])
            if (st, mt) == (3, 1):
                nc.gpsimd.dma_start(
                    T["out_p"][sb * 512 : (sb + 1) * 512, :].rearrange(
                        "(t p) m -> p t m", p=128
                    ),
                    ot[:].rearrange("p (t m) -> p t m", t=4),
                )

        def emit_norm(sb, pair, po2):
            # po2 rows 0-63 hold U (unnormalized out), rows 64-127 hold the
            # denominator replicated 64x (from vpc's ones-columns), so one
            # lane-parallel reciprocal + one multiply normalizes a head.
            sl = slice(sb * 512, (sb + 1) * 512)
            for h2 in range(2):
                h = pair * 2 + h2
                psl = slice(h2 * 64, h2 * 64 + 64)
                po = po2[h2]
                rcs = smallp.tile([64, 512], f32, tag="rcs", name=f"rcs{sb}_{h}")
                nc.vector.tensor_copy(rcs[:], po[64:128, :])
                rc = smallp.tile([64, 512], f32, tag="rc", name=f"rc{sb}_{h}")
                nc.vector.reciprocal_approx_fast(rc[:], rcs[:])
                nc.vector.tensor_mul(
                    concatT[pair][psl, sl], po[0:64, :], rc[:]
                )

        # ---- static extras schedule ----
        # extras[it][c] -> list of thunks, emitted after that chunk's
        # scores+exp+attnV.  Placement is deadline-driven: a producer must be
        # EMITTED strictly before the first chunk whose instructions consume
        # it (the PE queue is in-order; a consumer emitted earlier would
        # head-of-line block on data its own queue never produces).
        extras = {it: {} for it in range(8)}

        def sched(it, c, fn):
            extras[it].setdefault(c, []).append(fn)

        # it0 (0,0): k projections (j0 feeds this iteration from chunk 4s';
        # j1 feeds it1), v projections (feed attnV during it1), q j1.
        sched(0, 1, lambda: emit_kproj_jt(1, 0))
        sched(0, 3, lambda: emit_kproj_jt(1, 1))
        sched(0, 5, lambda: emit_kproj_jt(2, 0))
        sched(0, 6, lambda: emit_vproj(0))
        sched(0, 7, lambda: emit_kproj_jt(2, 1))
        sched(0, 8, lambda: emit_kproj_jt(3, 0))
        sched(0, 9, lambda: emit_vproj(1))
        sched(0, 10, lambda: emit_vproj(2))
        sched(0, 11, lambda: emit_qproj_jt(0, 1))
        sched(0, 12, lambda: emit_kproj_jt(3, 1))
        sched(0, 13, lambda: emit_vproj(3))
        sched(0, 14, lambda: emit_vproj(4))
        sched(0, 14, lambda: emit_vdma(2))
        sched(0, 15, lambda: emit_vproj(5))
        # it1 (0,1): vproj 6..15, prefetch q(1)/mask(1)
        for i, cc in enumerate(range(1, 11)):
            sched(1, cc, lambda ch=6 + i: emit_vproj(ch))
        sched(1, 3, lambda: emit_vdma(3))
        sched(1, 8, lambda: emit_qdma(1))
        sched(1, 12, lambda: emit_qproj_jt(1, 0))
        sched(1, 14, lambda: emit_mask_dma(1, 0))
        sched(1, 15, lambda: emit_mask_dma(1, 1))
        # steady iterations
        for sb in range(1, NSB):
            it = 2 * sb
            sched(it, 0, lambda s=sb: emit_qproj_jt(s, 1))
            # wo for previous sb: its concatT is only complete after
            # norm(sb-1, 1), which runs at the END of iteration (sb, 0) —
            # so the wo groups go in iteration (sb, 1).
            for g in range(8):
                sched(it + 1, g,
                      lambda s=sb - 1, a=g // 2, b=g % 2: emit_wo_group(s, a, b))
            if sb + 1 < NSB:
                sched(it, 12, lambda s=sb + 1: emit_qdma(s))
                sched(it, 14, lambda s=sb + 1: emit_mask_dma(s, 0))
                sched(it, 15, lambda s=sb + 1: emit_mask_dma(s, 1))
                sched(it + 1, 12, lambda s=sb + 1: emit_qproj_jt(s, 0))

        # ---- prologue ----
        # The DMA engines ramp slowly from idle (~40 GB/s for the first
        # ~0.5MB), so each queue leads with a throwaway transfer.  The
        # critical path (wkq combo + k0/q0 on the scalar HWDGE ring) then
        # runs on warmed engines; wv/v0/v1/wo ride the sync queue.
        dwarm = persist.tile([128, 512], bf, tag="dwarm")
        nc.gpsimd.dma_start(
            dwarm[:, 0:256], T["qT"][0, :, 0, 0:256]
        )
        nc.sync.dma_start(
            dwarm[:, 256:512], T["qT"][0, :, 1, 0:256]
        )
        nc.sync.dma_start(biasqk[:], T["biasqk"][:, :])
        emit_kdma(0, nc.sync)
        emit_wdma(wkq, "wkq", nc.gpsimd)
        emit_qdma(0, nc.gpsimd)
        emit_kdma(1, nc.sync)
        emit_kdma(2)
        emit_wdma(wv, "wvT", nc.sync)
        emit_vdma(0, nc.sync)
        emit_vdma(1, nc.sync)
        emit_wodma(0)
        emit_wodma(1)
        # HAM warm-up: dummy matmuls bridge the PE from t~6.5us until the
        # first projection inputs land (~20us), so nothing runs at the cold
        # 1.2 GHz clock.  memset on DVE: the gpsimd queue is busy with
        # DMA descriptor generation at t=0.
        warm = persist.tile([128, 512], bf, tag="warm")
        nc.vector.memset(warm[:], 0.0)
        wps = bigp.tile([128, 512], f32, tag="big", name="warmps")
        for i in range(36):
            nc.tensor.matmul(
                wps[:], warm[:, 0:128], warm[:],
                start=(i == 0), stop=(i == 35),
            )
        emit_kproj_jt(0, 0)
        emit_kproj_jt(0, 1)
        emit_qproj_jt(0, 0)
        emit_mask_dma(0, 0)
        emit_mask_dma(0, 1)
        # kTt(3) recycles kTt(0)'s buffer, so its DMA instruction carries a
        # sem-wait on kp(0,*) that would head-of-line block the gpsimd DMA
        # ring — it goes last, after the masks.
        emit_kdma(3)

        # ---- main pipeline ----
        # Per chunk: attnV(i-1) first (deps always stale -> PE never
        # head-of-line blocks), then scores(i) (waits only on the exp two
        # chunks back), then exp on ACT.  Extras fill the remaining PE slack.
        po2L = None
        prev = None        # (sb, pair, Pt)
        for sb in range(NSB):
            for pair in range(2):
                it = 2 * sb + pair
                last_it = (sb == NSB - 1 and pair == 1)

                Pt = ptp.tile(
                    [128, 2 * NC_T * 512], bf, tag="Pt", name=f"Pt{sb}_{pair}"
                )
                pv = Pt[:].rearrange("p (c h s) -> p c h s", c=NC_T, h=2)
                if prev is not None:
                    po2 = [
                        bigp.tile([128, 512], f32, tag="big",
                                  name=f"av{prev[0]}_{prev[1]}_{h2}")
                        for h2 in range(2)
                    ]
                def attn_v(dst, src_pt, c, h2, start, stop):
                    h = src_pt[1] * 2 + h2
                    nc.tensor.matmul(
                        dst[h2][:],
                        vpc[c][:, h * 128 : h * 128 + 128],
                        src_pt[2][:, (2 * c + h2) * 512 : (2 * c + h2 + 1) * 512],
                        start=start, stop=stop,
                    )

                def mask_mul(chunks):
                    hf, q = chunks[0] // 8, slice(chunks[0] % 8, chunks[0] % 8 + len(chunks))
                    mv = mtiles[(sb, hf)][:].rearrange("p (c s) -> p c s", c=8)
                    csl = slice(chunks[0], chunks[-1] + 1)
                    for h2 in range(2):
                        nc.vector.tensor_mul(
                            pv[:, csl, h2, :], pv[:, csl, h2, :], mv[:, q, :]
                        )

                cur = (sb, pair, Pt)
                for c in range(NC_T):
                    if prev is not None:
                        for h2 in range(2):
                            attn_v(po2, prev, c, h2, c == 0, c == NC_T - 1)
                    ps = scp.tile(
                        [128, 1024], f32, tag="sc", name=f"sc{sb}_{pair}_{c}"
                    )
                    for h2 in range(2):
                        psl = slice(h2 * 64, h2 * 64 + 64)
                        nc.tensor.matmul(
                            ps[:, h2 * 512 : (h2 + 1) * 512],
                            kpT[pair][c // 8][psl, (c % 8) * 128 : (c % 8 + 1) * 128],
                            qpS[pair][sb][psl, :],
                            start=True, stop=True,
                        )
                    nc.scalar.activation(
                        Pt[:, c * 1024 : (c + 1) * 1024],
                        ps[:], Exp, scale=SCALE,
                    )
                    if last_it and c >= 8:
                        # last iteration: drain our own attnV early so the
                        # tail after the final exp is as short as possible
                        if c == 8:
                            po2L = [
                                bigp.tile([128, 512], f32, tag="big",
                                          name=f"avL_{h2}")
                                for h2 in range(2)
                            ]
                        for h2 in range(2):
                            attn_v(po2L, cur, c - 8, h2, c == 8, False)
                        if c >= 12:
                            for h2 in range(2):
                                attn_v(po2L, cur, c - 4, h2, False, False)
                    if last_it:
                        if c in (7, 11, 13, 15):
                            mask_mul({7: list(range(0, 8)), 11: [8, 9, 10, 11],
                                      13: [12, 13], 15: [14, 15]}[c])
                    elif c == 7 or c == NC_T - 1:
                        mask_mul(list(range(0, 8)) if c == 7 else
                                 list(range(8, NC_T)))
                    for fn in extras[it].get(c, ()):
                        fn()
                if prev is not None:
                    emit_norm(prev[0], prev[1], po2)
                prev = cur
        # tail: finish attnv(3,1) chunks 12..15, then norm + final Wo
        psb, ppair, pPt = prev
        for c in range(12, NC_T):
            for h2 in range(2):
                attn_v(po2L, prev, c, h2, False, c == NC_T - 1)
        emit_norm(psb, ppair, po2L)
        for st in range(4):
            for mt in range(2):
                emit_wo_group(NSB - 1, st, mt)


def build_nc():
    nc = bacc.Bacc("TRN2", target_bir_lowering=False, debug=False)
    names = {}
    def din(name, shape, dt):
        names[name] = nc.dram_tensor(name, shape, dt, kind="ExternalInput").ap()
    # q/k/v pre-tiled on host to [sb, p, c, s] and mask to [sb, p, c, s] so
    # every DMA descriptor covers a full 8-16KB partition line (the
    # descriptor-generation rate, ~12ns/descriptor, caps DMA throughput
    # otherwise).
    din("qT", [NSB, 128, 8, 512], bf)
    din("kT", [NSB, 128, 8, 512], bf)
    din("vT", [NSB, 128, 8, 512], bf)
    din("maskT", [NSB, 128, NC_T, 512], bf)
    din("wkq", [128, 8, 2 * JC], bf)
    din("wvT", [128, 8, JC], bf)
    din("woT", [JC, D], bf)
    din("biasqk", [128, 4], f32)
    names["out_p"] = nc.dram_tensor(
        "out_p", [S, D], bf, kind="ExternalOutput"
    ).ap()
    with tile_mod.TileContext(nc) as tc:
        _emit(tc, names)
    nc.compile()
    return nc


_NC = None


def _tile_ds(xT, nc_):
    """[D, S] -> [NSB, 128, nc_, S // nc_ // ...] host pre-tiling.

    Element (sb, p, c, s) = xT[c * 128 + p, sb * blk + s] where blk = S/NSB.
    """
    d, s_ = xT.shape
    blk = s_ // NSB
    nch = d // 128
    # xT[(c p), (sb s)] -> [c, p, sb, s] -> [sb, p, c, s]
    r = xT.reshape(nch, 128, NSB, blk).transpose(2, 1, 0, 3)
    return np.ascontiguousarray(r)


def prep_inputs(q, k, v, mask, Wq, bq, Wk, bk, Wv, bv, Wo, bo):
    q = np.asarray(q, F32)
    k = np.asarray(k, F32)
    v = np.asarray(v, F32)
    mask = np.asarray(mask)
    Wq, Wk, Wv, Wo = (np.asarray(w, F32) for w in (Wq, Wk, Wv, Wo))
    bq, bk, bv, bo = (np.asarray(b_, F32) for b_ in (bq, bk, bv, bo))

    maskT = _tile_ds(np.ascontiguousarray(mask[0, 0].T).astype(BF16), NC_T)
    qT = [_tile_ds(q[b_].T.astype(BF16), 8) for b_ in range(B)]
    kT = [_tile_ds(k[b_].T.astype(BF16), 8) for b_ in range(B)]
    vT = [_tile_ds(v[b_].T.astype(BF16), 8) for b_ in range(B)]

    def _tile_w(wT):
        # [D, JC] -> [128, 8, JC]
        return np.ascontiguousarray(
            wT.reshape(8, 128, JC).transpose(1, 0, 2)
        )

    in_maps = []
    for c in range(N_CORES):
        b_, g = c // 4, c % 4
        js = slice(g * JC, (g + 1) * JC)
        biasqk = np.stack(
            [bq[js][:128], bq[js][128:], bk[js][:128], bk[js][128:]], axis=1
        ).astype(F32)
        in_maps.append(
            {
                "qT": qT[b_],
                "kT": kT[b_],
                "vT": vT[b_],
                "maskT": maskT,
                "wkq": np.ascontiguousarray(np.concatenate(
                    [_tile_w(Wk[js, :].T.astype(BF16)),
                     _tile_w(Wq[js, :].T.astype(BF16))], axis=2)),
                "wvT": _tile_w(Wv[js, :].T.astype(BF16)),
                "woT": np.ascontiguousarray(Wo[:, js].T).astype(BF16),
                "biasqk": np.ascontiguousarray(biasqk),
            }
        )
    # bv contributes a constant (softmax rows sum to 1): out += Wo @ bv + bo
    bias_out = (Wo @ bv + bo).astype(F32)
    return in_maps, bias_out


def run_prepped(in_maps, bias_out, trace=False, **kw):
    global _NC
    if _NC is None:
        _NC = build_nc()
    res = run_bass_kernel_spmd(
        _NC, in_maps, list(range(N_CORES)), trace=trace, **kw
    )
    out = np.zeros((B, S, D), F32)
    for c in range(N_CORES):
        out[c // 4] += res.results[c]["out_p"].astype(F32)
    out += bias_out[None, None, :]
    return out, res


def kernel(q, k, v, mask, Wq, bq, Wk, bk, Wv, bv, Wo, bo):
    in_maps, bias_out = prep_inputs(
        q, k, v, mask, Wq, bq, Wk, bk, Wv, bv, Wo, bo
    )
    out, _ = run_prepped(in_maps, bias_out)
    return out

